# revision 23
# baseline (speedup 1.0000x reference)
"""Trainium2 Bass kernel for nn_ALTER2Layer (dense_mlp, 8-core data parallel).

Math per batch b:
  c1 = sig(x W1^T + b1); c2 = sig(c1 W2^T + b2); c3 = sig(c2 W2 + b3)
  r  = sig(c3 W1 + b_r)
  s_i = c_i (1 - c_i)
  J[b] = W1^T D1 W2^T D2 W2 D3 W1          (D_i = diag(s_i))
Factored on device as:
  V = D2 W2 D3 W1          [32, 1024]
  Q = D1 W2^T V            [64, 1024]
  J = W1^T Q               [1024, 1024]
All transposes of weights/inputs and the reference's batch-interleave
reshape of the Jacobians are done host-side (free; only NEFF time counts).

Sharding: pure data parallel, batch dim 128 -> 16 per core across 8 cores.
Each core processes 48 (input, batch) pairs: [x | x_noise | z] x 16.
"""

import os
import numpy as np

B = 128
D = 1024
C0 = 64
C1 = 32
NCORES = 8
BS = B // NCORES          # batches per core = 16
NPAIR = 3 * BS            # (input, batch) pairs per core = 48
KT = D // 128             # 8 k-tiles of 128

LAST_RESULT = None        # BassKernelResults of the most recent run (for test.py)


def _build_nc():
    import concourse.bass as bass
    import concourse.mybir as mybir
    from concourse import bacc
    from concourse.tile import TileContext, add_dep_helper

    f32 = mybir.dt.float32
    f32r = mybir.dt.float32r
    bf16 = mybir.dt.bfloat16
    AF = mybir.ActivationFunctionType

    nc = bacc.Bacc()

    # fp32 self-loading matmuls have a single HW sync-wait slot, so every
    # real matmul must carry <=1 semaphore wait. Dummy 1x1 "absorber"
    # matmuls read one foreign-engine-produced tile each; Tile's tracker
    # then treats those semaphore ticks as observed by PE.
    _scratch = {"pool": None}

    def absorb(ap):
        scr = _scratch["pool"].tile([1, 1], mybir.dt.float32, tag="ps", name="scr")
        mm = nc.tensor.matmul(scr[0:1, 0:1], lhsT=ap, rhs=ap,
                              start=True, stop=True)
        return mm

    def order_after(inst, fence):
        if fence is not None:
            add_dep_helper(inst.ins, fence.ins, sync=False, reason="fence order")

    # ---- DRAM parameters (per-core shards / replicated weights) ----
    xt3_d = nc.dram_tensor("xt3", [128, KT * NPAIR], f32, kind="ExternalInput")
    w1_d = nc.dram_tensor("w1", [C0, D], bf16, kind="ExternalInput")
    w1t_d = nc.dram_tensor("w1t", [128, KT * C0], f32, kind="ExternalInput")
    w1a_d = nc.dram_tensor("w1a", [C0 + 1, D], f32, kind="ExternalInput")
    w2_d = nc.dram_tensor("w2", [C1, C0], f32, kind="ExternalInput")
    w2r_d = nc.dram_tensor("w2r", [C1, C0], bf16, kind="ExternalInput")
    w2t_d = nc.dram_tensor("w2t", [C0, C1], f32, kind="ExternalInput")
    b1_d = nc.dram_tensor("b1c", [C0, 1], f32, kind="ExternalInput")
    b2_d = nc.dram_tensor("b2c", [128, 1], f32, kind="ExternalInput")
    b3_d = nc.dram_tensor("b3c", [C0, 1], f32, kind="ExternalInput")

    out_r = nc.dram_tensor("out_r", [BS, D], f32, kind="ExternalOutput")
    out_c2t = nc.dram_tensor("out_c2t", [C1, BS], f32, kind="ExternalOutput")
    # 48 Jacobians stacked: rows [p*1024, (p+1)*1024) = J of pair p
    out_j = nc.dram_tensor("out_j", [NPAIR * D, D], bf16, kind="ExternalOutput")

    from contextlib import ExitStack

    with TileContext(nc) as tc, ExitStack() as stk:
        const = stk.enter_context(tc.tile_pool(name="const", bufs=1))
        enc = stk.enter_context(tc.tile_pool(name="enc", bufs=1))
        uv = stk.enter_context(tc.tile_pool(name="uv", bufs=3))
        jst = stk.enter_context(tc.tile_pool(name="jst", bufs=3))
        ps_a = stk.enter_context(tc.tile_pool(name="ps_a", bufs=2, space="PSUM"))
        ps_j = stk.enter_context(tc.tile_pool(name="ps_j", bufs=3, space="PSUM"))
        _scratch["pool"] = ps_a

        # ---- load constants ----
        dmas = []
        xt_sb = const.tile([128, KT * NPAIR], f32)
        dmas.append(nc.sync.dma_start(out=xt_sb[:, :], in_=xt3_d[:, :]))
        w1_sb = const.tile([C0, D], bf16)
        dmas.append(nc.sync.dma_start(out=w1_sb[:, :], in_=w1_d[:, :]))
        w1t_sb = const.tile([128, KT * C0], f32)
        dmas.append(nc.sync.dma_start(out=w1t_sb[:, :], in_=w1t_d[:, :]))
        w1a_sb = const.tile([C0 + 1, D], f32)
        dmas.append(nc.sync.dma_start(out=w1a_sb[:, :], in_=w1a_d[:, :]))
        w2_sb = const.tile([C1, C0], f32)
        dmas.append(nc.sync.dma_start(out=w2_sb[:, :], in_=w2_d[:, :]))
        w2r_sb = const.tile([C1, C0], bf16)
        dmas.append(nc.sync.dma_start(out=w2r_sb[:, :], in_=w2r_d[:, :]))
        w2t_sb = const.tile([C0, C1], f32)
        dmas.append(nc.sync.dma_start(out=w2t_sb[:, :], in_=w2t_d[:, :]))
        b1_sb = const.tile([C0, 1], f32)
        dmas.append(nc.sync.dma_start(out=b1_sb[:, :], in_=b1_d[:, :]))
        b2q_sb = const.tile([128, 1], f32)
        dmas.append(nc.sync.dma_start(out=b2q_sb[:, :], in_=b2_d[:, :]))
        b3_sb = const.tile([C0, 1], f32)
        dmas.append(nc.sync.dma_start(out=b3_sb[:, :], in_=b3_d[:, :]))

        fence0 = None
        for t in (xt_sb, w1_sb, w1t_sb, w1a_sb, w2_sb, w2t_sb,
                  b1_sb, b2q_sb, b3_sb):
            fence0 = absorb(t[0:1, 0:1])
        # HAM warm-up: ~7us of dense matmuls so the PE clock-gate opens
        # (K=8/8 @ 2.4GHz) before the real work starts.
        for w in range(16):
            pj = ps_j.tile([128, D], f32, tag="pj", name="warm")
            nc.tensor.matmul(pj[:, 0:512], lhsT=w1_sb[:, 0:128],
                             rhs=w1_sb[:, 0:512], start=True, stop=True)

        # ---- encode all 48 pairs at once (feature-on-partition layouts) ----
        # c1t[c, p] = sig(sum_d W1[c,d] X[p,d] + b1[c])
        c1_ps = ps_a.tile([C0, NPAIR], f32, tag="ps")
        for k in range(KT):
            mm = nc.tensor.matmul(
                c1_ps[:, :],
                lhsT=w1t_sb[:, k * C0:(k + 1) * C0],
                rhs=xt_sb[:, k * NPAIR:(k + 1) * NPAIR],
                start=(k == 0),
                stop=(k == KT - 1),
            )
            if k == 0:
                order_after(mm, fence0)
        c1t = enc.tile([C0, NPAIR], f32)
        nc.scalar.activation(c1t[:, :], c1_ps[:, :], AF.Sigmoid, bias=b1_sb[:, :])
        om1 = enc.tile([C0, NPAIR], f32, tag="om")
        nc.scalar.activation(om1[:, :], c1t[:, :], AF.Copy, bias=1.0, scale=-1.0)
        s1t = enc.tile([C0, NPAIR], f32)
        nc.vector.tensor_mul(s1t[:, :], c1t[:, :], om1[:, :])

        # c2t[j, p] = sig(sum_c W2[j,c] c1t[c,p] + b2[j]); col-packed x4 so the
        # sigmoid/derivative comes out replicated on all four partition groups
        c2_ps = ps_a.tile([128, NPAIR], f32, tag="ps")
        for j in range(4):
            nc.tensor.matmul(c2_ps[j * C1:(j + 1) * C1, :], lhsT=w2t_sb[:, :],
                             rhs=c1t[:, :], start=True, stop=True,
                             tile_position=(0, j * C1))
        c2t4 = enc.tile([128, NPAIR], f32)
        nc.scalar.activation(c2t4[:, :], c2_ps[:, :], AF.Sigmoid,
                             bias=b2q_sb[:, :])
        c2t = c2t4
        om2 = enc.tile([128, NPAIR], f32, tag="om2")
        nc.scalar.activation(om2[:, :], c2t4[:, :], AF.Copy, bias=1.0, scale=-1.0)
        s2t4 = enc.tile([128, NPAIR], f32)
        nc.vector.tensor_mul(s2t4[:, :], c2t4[:, :], om2[:, :])

        # c3t[c, p] = sig(sum_j W2[j,c] c2t[j,p] + b3[c]); keep a ones row under it
        fence_c3 = absorb(s2t4[0:1, 0:1])
        c3_ps = ps_a.tile([C0, NPAIR], f32, tag="ps")
        mm = nc.tensor.matmul(c3_ps[:, :], lhsT=w2_sb[:, :], rhs=c2t[:C1, :],
                              start=True, stop=True)
        order_after(mm, fence_c3)
        c3ta = enc.tile([C0 + 1, NPAIR], f32)
        nc.scalar.activation(c3ta[:C0, :], c3_ps[:, :], AF.Sigmoid, bias=b3_sb[:, :])
        ones_set = nc.vector.memset(c3ta[C0:C0 + 1, :], 1.0)
        om3 = enc.tile([C0, NPAIR], f32, tag="om")
        nc.scalar.activation(om3[:, :], c3ta[:C0, :], AF.Copy, bias=1.0, scale=-1.0)
        s3t = enc.tile([C0, NPAIR], f32)
        nc.vector.tensor_mul(s3t[:, :], c3ta[:C0, :], om3[:, :])

        # recover (x pairs only, cols 0:BS): r = sig(c3 W1 + b_r) via W1-augmented
        fence_r = absorb(c3ta[C0:C0 + 1, 0:1])
        r_sb = enc.tile([BS, D], f32)
        for h in range(2):
            r_ps = ps_a.tile([BS, 512], f32, tag="ps")
            mm = nc.tensor.matmul(
                r_ps[:, :],
                lhsT=c3ta[:, 0:BS],
                rhs=w1a_sb[:, h * 512:(h + 1) * 512],
                start=True, stop=True,
            )
            if h == 0:
                order_after(mm, fence_r)
            nc.scalar.activation(r_sb[:, h * 512:(h + 1) * 512], r_ps[:, :],
                                 AF.Sigmoid)
        nc.sync.dma_start(out=out_r[:, :], in_=r_sb[:, :])
        nc.sync.dma_start(out=out_c2t[:, :], in_=c2t[:C1, 0:BS])

        # ---- per-pair Jacobian ----
        # K=32 factorization: UT = D2 W2 D1 W1 [32,1024], V = W2 D3 W1 [32,1024],
        # J = UT^T V. UT/V are built 4x-replicated across partition groups by
        # col-tiled matmuls; J then issues 4 concurrent row-group matmuls
        # (the 128x128 PE array runs all four 32-row tiles at once).
        for p in range(NPAIR):
            w2t_s1 = uv.tile([C0, 2 * C1], bf16, tag="w2ts")
            nc.vector.tensor_scalar_mul(w2t_s1[:, 0:C1], w2t_sb[:, :],
                                        s1t[:, p:p + 1])
            nc.vector.tensor_scalar_mul(w2t_s1[:, C1:2 * C1], w2t_sb[:, :],
                                        s3t[:, p:p + 1])

            ut4 = uv.tile([128, D], bf16, tag="ut4")
            v4 = uv.tile([128, D], bf16, tag="v4")
            for h in range(2):
                ut_ps = ps_a.tile([128, 512], f32, tag="ps", name="ut_ps")
                for j in range(4):
                    nc.tensor.matmul(
                        ut_ps[j * C1:(j + 1) * C1, :],
                        lhsT=w2t_s1[:, 0:C1],
                        rhs=w1_sb[:, h * 512:(h + 1) * 512],
                        start=True, stop=True, tile_position=(0, j * C1))
                nc.scalar.activation(ut4[:, h * 512:(h + 1) * 512], ut_ps[:, :],
                                     AF.Copy, scale=s2t4[:, p:p + 1])
                v_ps = ps_a.tile([128, 512], f32, tag="ps", name="v_ps")
                for j in range(4):
                    nc.tensor.matmul(
                        v_ps[j * C1:(j + 1) * C1, :],
                        lhsT=w2t_s1[:, C1:2 * C1],
                        rhs=w1_sb[:, h * 512:(h + 1) * 512],
                        start=True, stop=True, tile_position=(0, j * C1))
                nc.scalar.copy(v4[:, h * 512:(h + 1) * 512], v_ps[:, :])

            # J = UT^T V: 8 i-tiles in 2 groups of 4 concurrent row-group MMs
            fence_j = absorb(ut4[0:1, 0:1])
            fence_j2 = absorb(v4[0:1, 0:1])
            order_after(fence_j2, fence_j)
            jstage = jst.tile([128, KT * D], bf16, tag="jst")
            ncopy = 0
            for grp in range(2):
                for h in range(2):
                    tiles = []
                    for g in range(4):
                        i = grp * 4 + g
                        pj = ps_j.tile([128, 512], f32, tag="pj")
                        mm = nc.tensor.matmul(
                            pj[:, :],
                            lhsT=ut4[g * C1:(g + 1) * C1, i * 128:(i + 1) * 128],
                            rhs=v4[g * C1:(g + 1) * C1, h * 512:(h + 1) * 512],
                            start=True, stop=True, tile_position=(g * C1, 0))
                        if grp == 0 and h == 0 and g == 0:
                            order_after(mm, fence_j2)
                        tiles.append((i, pj))
                    for i, pj in tiles:
                        dst = jstage[:, i * D + h * 512:i * D + (h + 1) * 512]
                        if ncopy % 16 < 9:
                            nc.vector.tensor_copy(dst, pj[:, :])
                        else:
                            nc.scalar.copy(dst, pj[:, :])
                        ncopy += 1
            # one 2MB DMA: dest rows p*1024 + i*128 + q are contiguous per i
            dst = out_j[p * D:(p + 1) * D, :].rearrange("(t q) d -> q t d", q=128)
            src2 = jstage[:, :].rearrange("q (t d) -> q t d", t=KT)
            nc.sync.dma_start(out=dst, in_=src2)

    nc.compile()
    return nc


def _host_inputs(x, x_noise, z, W1, b1, W2, b2, b3, b_r):
    """Build per-core input maps (all host-side transposes happen here)."""
    import ml_dtypes
    w1 = np.ascontiguousarray(W1).astype(ml_dtypes.bfloat16)
    w1t = np.ascontiguousarray(
        W1.T.reshape(KT, 128, C0).transpose(1, 0, 2).reshape(128, KT * C0),
        dtype=np.float32)
    w1a = np.concatenate([W1, b_r[None, :]], axis=0).astype(np.float32)
    w2 = np.ascontiguousarray(W2, dtype=np.float32)
    w2t = np.ascontiguousarray(W2.T, dtype=np.float32)
    b1c = np.ascontiguousarray(b1[:, None], dtype=np.float32)
    b2c = np.ascontiguousarray(np.tile(b2, 4)[:, None], dtype=np.float32)
    b3c = np.ascontiguousarray(b3[:, None], dtype=np.float32)

    in_maps = []
    for c in range(NCORES):
        sl = slice(c * BS, (c + 1) * BS)
        X = np.concatenate([x[sl], x_noise[sl], z[sl]], axis=0)  # [48, 1024]
        xt3 = np.ascontiguousarray(
            X.T.reshape(KT, 128, NPAIR).transpose(1, 0, 2).reshape(128, KT * NPAIR),
            dtype=np.float32)
        in_maps.append({
            "xt3": xt3, "w1": w1, "w1t": w1t, "w1a": w1a,
            "w2": w2, "w2r": w2.astype(ml_dtypes.bfloat16), "w2t": w2t,
            "b1c": b1c, "b2c": b2c, "b3c": b3c,
        })
    return in_maps


def kernel(x, x_noise, z, W1, b1, W2, b2, b3, b_r):
    global LAST_RESULT
    from concourse.bass_utils import run_bass_kernel_spmd

    x = np.asarray(x, dtype=np.float32)
    x_noise = np.asarray(x_noise, dtype=np.float32)
    z = np.asarray(z, dtype=np.float32)
    W1 = np.asarray(W1, dtype=np.float32)
    b1 = np.asarray(b1, dtype=np.float32)
    W2 = np.asarray(W2, dtype=np.float32)
    b2 = np.asarray(b2, dtype=np.float32)
    b3 = np.asarray(b3, dtype=np.float32)
    b_r = np.asarray(b_r, dtype=np.float32)

    nc = _build_nc()
    in_maps = _host_inputs(x, x_noise, z, W1, b1, W2, b2, b3, b_r)
    trace = bool(int(os.environ.get("KERNEL_TRACE", "0")))
    res = run_bass_kernel_spmd(nc, in_maps, core_ids=list(range(NCORES)),
                               trace=trace)
    LAST_RESULT = res
    results = res.results

    recover = np.concatenate([results[c]["out_r"] for c in range(NCORES)], axis=0)
    c2 = np.concatenate([results[c]["out_c2t"].T for c in range(NCORES)], axis=0)

    jacs = []
    for which in range(3):  # x, x_noise, z
        J = np.empty((B, D, D), dtype=np.float32)
        for c in range(NCORES):
            blk = np.asarray(results[c]["out_j"]).astype(np.float32)
            blk = blk.reshape(3, BS, D, D)[which]
            J[c * BS:(c + 1) * BS] = blk
        # reference's cat(dim=1)+reshape batch scrambling
        J = J.transpose(1, 0, 2).reshape(D, B * D).reshape(B, D, D)
        jacs.append(J)

    return (recover, c2, jacs[0], jacs[1], jacs[2])


# revision 24
# speedup vs baseline: 1.0058x; 1.0058x over previous
"""Trainium2 Bass kernel for nn_ALTER2Layer (dense_mlp, 8-core data parallel).

Math per batch b:
  c1 = sig(x W1^T + b1); c2 = sig(c1 W2^T + b2); c3 = sig(c2 W2 + b3)
  r  = sig(c3 W1 + b_r)
  s_i = c_i (1 - c_i)
  J[b] = W1^T D1 W2^T D2 W2 D3 W1          (D_i = diag(s_i))
Factored on device as (bf16 operands, fp32 PSUM accumulation):
  V  = D2 W2 D3 W1         [32, 1024]
  Q  = D1 W2^T V           [64, 1024]   (built twice, rows 0-63 and 64-127,
                                         by col-tiled matmuls sharing one rhs)
  J  = W1^T Q              [1024, 1024] (i-tiles alternate PE row halves so
                                         each matmul's weight load overlaps
                                         the previous matmul's stream)
The Jacobian is staged in bf16 (halves HBM write traffic); the host widens
back to fp32. All weight/input transposes and the reference's
batch-interleave reshape of the Jacobians happen host-side.

Sharding: pure data parallel, batch dim 128 -> 16 per core across 8 cores.
Each core processes 48 (input, batch) pairs: [x | x_noise | z] x 16.
"""

import os
import numpy as np

B = 128
D = 1024
C0 = 64
C1 = 32
NCORES = 8
BS = B // NCORES          # batches per core = 16
NPAIR = 3 * BS            # (input, batch) pairs per core = 48
KT = D // 128             # 8 k-tiles of 128

LAST_RESULT = None        # BassKernelResults of the most recent run (for test.py)


def _build_nc():
    import concourse.bass as bass
    import concourse.mybir as mybir
    from concourse import bacc
    from concourse.tile import TileContext, add_dep_helper
    from contextlib import ExitStack

    f32 = mybir.dt.float32
    bf16 = mybir.dt.bfloat16
    AF = mybir.ActivationFunctionType

    nc = bacc.Bacc()

    # Dummy 1x1 "absorber" matmuls read one foreign-engine-produced tile
    # each, so the following real matmuls carry few semaphore waits.
    _scratch = {"pool": None}

    def absorb(ap):
        scr = _scratch["pool"].tile([1, 1], f32, tag="ps", name="scr")
        return nc.tensor.matmul(scr[0:1, 0:1], lhsT=ap, rhs=ap,
                                start=True, stop=True)

    def order_after(inst, fence):
        if fence is not None:
            add_dep_helper(inst.ins, fence.ins, sync=False, reason="fence order")

    # ---- DRAM parameters (per-core shards / replicated weights) ----
    xt3_d = nc.dram_tensor("xt3", [128, KT * NPAIR], f32, kind="ExternalInput")
    # W1 stacked twice on the partition axis (rows 0-63 == rows 64-127)
    w1_d = nc.dram_tensor("w1", [128, D], bf16, kind="ExternalInput")
    w1t_d = nc.dram_tensor("w1t", [128, KT * C0], f32, kind="ExternalInput")
    w1a_d = nc.dram_tensor("w1a", [C0 + 1, D], f32, kind="ExternalInput")
    w2_d = nc.dram_tensor("w2", [C1, C0], f32, kind="ExternalInput")
    w2r_d = nc.dram_tensor("w2r", [C1, C0], bf16, kind="ExternalInput")
    w2t_d = nc.dram_tensor("w2t", [C0, C1], f32, kind="ExternalInput")
    b1_d = nc.dram_tensor("b1c", [128, 1], f32, kind="ExternalInput")
    b2_d = nc.dram_tensor("b2c", [C1, 1], f32, kind="ExternalInput")
    b3_d = nc.dram_tensor("b3c", [C0, 1], f32, kind="ExternalInput")

    out_r = nc.dram_tensor("out_r", [BS, D], f32, kind="ExternalOutput")
    out_c2t = nc.dram_tensor("out_c2t", [C1, BS], f32, kind="ExternalOutput")
    # 48 Jacobians stacked: rows [p*1024, (p+1)*1024) = J of pair p (bf16)
    out_j = nc.dram_tensor("out_j", [NPAIR * D, D], bf16, kind="ExternalOutput")

    with TileContext(nc) as tc, ExitStack() as stk:
        const = stk.enter_context(tc.tile_pool(name="const", bufs=1))
        enc = stk.enter_context(tc.tile_pool(name="enc", bufs=1))
        uv = stk.enter_context(tc.tile_pool(name="uv", bufs=3))
        jst = stk.enter_context(tc.tile_pool(name="jst", bufs=3))
        ps_a = stk.enter_context(tc.tile_pool(name="ps_a", bufs=2, space="PSUM"))
        ps_j = stk.enter_context(tc.tile_pool(name="ps_j", bufs=3, space="PSUM"))
        _scratch["pool"] = ps_a

        # ---- load constants ----
        xt_sb = const.tile([128, KT * NPAIR], f32)
        nc.sync.dma_start(out=xt_sb[:, :], in_=xt3_d[:, :])
        w1_sb = const.tile([128, D], bf16)
        nc.sync.dma_start(out=w1_sb[:, :], in_=w1_d[:, :])
        w1t_sb = const.tile([128, KT * C0], f32)
        nc.sync.dma_start(out=w1t_sb[:, :], in_=w1t_d[:, :])
        w1a_sb = const.tile([C0 + 1, D], f32)
        nc.sync.dma_start(out=w1a_sb[:, :], in_=w1a_d[:, :])
        w2_sb = const.tile([C1, C0], f32)
        nc.sync.dma_start(out=w2_sb[:, :], in_=w2_d[:, :])
        w2r_sb = const.tile([C1, C0], bf16)
        nc.sync.dma_start(out=w2r_sb[:, :], in_=w2r_d[:, :])
        w2t_sb = const.tile([C0, C1], f32)
        nc.sync.dma_start(out=w2t_sb[:, :], in_=w2t_d[:, :])
        b1_sb = const.tile([128, 1], f32)
        nc.sync.dma_start(out=b1_sb[:, :], in_=b1_d[:, :])
        b2_sb = const.tile([C1, 1], f32)
        nc.sync.dma_start(out=b2_sb[:, :], in_=b2_d[:, :])
        b3_sb = const.tile([C0, 1], f32)
        nc.sync.dma_start(out=b3_sb[:, :], in_=b3_d[:, :])

        fence0 = None
        for t in (xt_sb, w1_sb, w1t_sb, w1a_sb, w2_sb, w2r_sb, w2t_sb,
                  b1_sb, b2_sb, b3_sb):
            fence0 = absorb(t[0:1, 0:1])

        # ---- encode all 48 pairs at once (feature-on-partition layouts) ----
        # c1, col-tiled x2 so s1 comes out replicated on both 64-row halves
        c1_ps = ps_a.tile([128, NPAIR], f32, tag="ps", name="c1_ps")
        for cg in (0, C0):
            for k in range(KT):
                mm = nc.tensor.matmul(
                    c1_ps[cg:cg + C0, :],
                    lhsT=w1t_sb[:, k * C0:(k + 1) * C0],
                    rhs=xt_sb[:, k * NPAIR:(k + 1) * NPAIR],
                    start=(k == 0), stop=(k == KT - 1),
                    tile_position=(0, cg),
                )
                if cg == 0 and k == 0:
                    order_after(mm, fence0)
        c1t2 = enc.tile([128, NPAIR], f32)
        nc.scalar.activation(c1t2[:, :], c1_ps[:, :], AF.Sigmoid, bias=b1_sb[:, :])
        om1 = enc.tile([128, NPAIR], f32, tag="om")
        nc.scalar.activation(om1[:, :], c1t2[:, :], AF.Copy, bias=1.0, scale=-1.0)
        s1t2 = enc.tile([128, NPAIR], f32)
        nc.vector.tensor_mul(s1t2[:, :], c1t2[:, :], om1[:, :])

        # c2t[j, p] = sig(sum_c W2[j,c] c1t[c,p] + b2[j])
        c2_ps = ps_a.tile([C1, NPAIR], f32, tag="ps", name="c2_ps")
        nc.tensor.matmul(c2_ps[:, :], lhsT=w2t_sb[:, :], rhs=c1t2[:C0, :],
                         start=True, stop=True)
        c2t = enc.tile([C1, NPAIR], f32)
        nc.scalar.activation(c2t[:, :], c2_ps[:, :], AF.Sigmoid, bias=b2_sb[:, :])
        om2 = enc.tile([128, NPAIR], f32, tag="om")
        nc.scalar.activation(om2[:C1, :], c2t[:, :], AF.Copy, bias=1.0, scale=-1.0)
        s2t = enc.tile([C1, NPAIR], f32)
        nc.vector.tensor_mul(s2t[:, :], c2t[:, :], om2[:C1, :])

        # c3t[c, p] = sig(sum_j W2[j,c] c2t[j,p] + b3[c]); ones row appended
        fence_c3 = absorb(s2t[0:1, 0:1])
        c3_ps = ps_a.tile([C0, NPAIR], f32, tag="ps", name="c3_ps")
        mm = nc.tensor.matmul(c3_ps[:, :], lhsT=w2_sb[:, :], rhs=c2t[:, :],
                              start=True, stop=True)
        order_after(mm, fence_c3)
        c3ta = enc.tile([C0 + 1, NPAIR], f32)
        nc.scalar.activation(c3ta[:C0, :], c3_ps[:, :], AF.Sigmoid, bias=b3_sb[:, :])
        nc.vector.memset(c3ta[C0:C0 + 1, :], 1.0)
        om3 = enc.tile([128, NPAIR], f32, tag="om")
        nc.scalar.activation(om3[:C0, :], c3ta[:C0, :], AF.Copy, bias=1.0, scale=-1.0)
        s3t = enc.tile([C0, NPAIR], f32)
        nc.vector.tensor_mul(s3t[:, :], c3ta[:C0, :], om3[:C0, :])

        # recover (x pairs only): r = sig(c3 W1 + b_r) via W1-augmented
        fence_r = absorb(c3ta[C0:C0 + 1, 0:1])
        r_sb = enc.tile([BS, D], f32)
        for h in range(2):
            r_ps = ps_a.tile([BS, 512], f32, tag="ps", name="r_ps")
            mm = nc.tensor.matmul(
                r_ps[:, :], lhsT=c3ta[:, 0:BS],
                rhs=w1a_sb[:, h * 512:(h + 1) * 512], start=True, stop=True)
            if h == 0:
                order_after(mm, fence_r)
            nc.scalar.activation(r_sb[:, h * 512:(h + 1) * 512], r_ps[:, :],
                                 AF.Sigmoid)
        nc.sync.dma_start(out=out_r[:, :], in_=r_sb[:, :])
        nc.sync.dma_start(out=out_c2t[:, :], in_=c2t[:, 0:BS])

        # ---- per-pair Jacobian ----
        for p in range(NPAIR):
            # lhsT for V: W2^T scaled by s3 along partitions (c)
            w2t_s3 = uv.tile([C0, C1], bf16, tag="w2ts3")
            nc.vector.tensor_scalar_mul(w2t_s3[:, :], w2t_sb[:, :],
                                        s3t[:, p:p + 1])
            # V = D2 W2 D3 W1  [32, 1024]; s2 applied on the PSUM drain
            v_sb = uv.tile([C1, D], bf16, tag="v")
            for h in range(2):
                v_ps = ps_a.tile([C1, 512], f32, tag="ps", name="v_ps")
                nc.tensor.matmul(v_ps[:, :], lhsT=w2t_s3[:, :],
                                 rhs=w1_sb[0:C0, h * 512:(h + 1) * 512],
                                 start=True, stop=True)
                nc.scalar.activation(v_sb[:, h * 512:(h + 1) * 512], v_ps[:, :],
                                     AF.Copy, scale=s2t[:, p:p + 1])

            # Q = D1 W2^T V, built on both 64-row halves by col-tiled matmuls
            # sharing the same rhs stream; s1 applied on the PSUM drain
            q2_sb = uv.tile([128, D], bf16, tag="q")
            for h in range(2):
                q_ps = ps_a.tile([128, 512], f32, tag="ps", name="q_ps")
                for cg in (0, C0):
                    nc.tensor.matmul(q_ps[cg:cg + C0, :], lhsT=w2r_sb[:, :],
                                     rhs=v_sb[:, h * 512:(h + 1) * 512],
                                     start=True, stop=True,
                                     tile_position=(0, cg))
                nc.scalar.activation(q2_sb[:, h * 512:(h + 1) * 512], q_ps[:, :],
                                     AF.Copy, scale=s1t2[:, p:p + 1])

            # J = W1^T Q: 8 i-tiles alternating PE row halves (weight loads
            # overlap the other half's stream); bf16 stage then one 2MB DMA
            fence_j = absorb(q2_sb[0:1, 0:1])
            jstage = jst.tile([128, KT * D], bf16, tag="jst")
            ncopy = 0
            for i in range(KT):
                rg = C0 if (i % 2) else 0
                pj = ps_j.tile([128, D], f32, tag="pj")
                for h in range(2):
                    mm = nc.tensor.matmul(
                        pj[:, h * 512:(h + 1) * 512],
                        lhsT=w1_sb[rg:rg + C0, i * 128:(i + 1) * 128],
                        rhs=q2_sb[rg:rg + C0, h * 512:(h + 1) * 512],
                        start=True, stop=True)
                    if i == 0 and h == 0:
                        order_after(mm, fence_j)
                # PSUM drains split across DVE/ACT
                for h in range(2):
                    dst = jstage[:, i * D + h * 512:i * D + (h + 1) * 512]
                    if ncopy % 16 < 9:
                        nc.vector.tensor_copy(dst, pj[:, h * 512:(h + 1) * 512])
                    else:
                        nc.scalar.copy(dst, pj[:, h * 512:(h + 1) * 512])
                    ncopy += 1
            # dest rows p*1024 + i*128 + q are contiguous (2KB runs) per i
            dst = out_j[p * D:(p + 1) * D, :].rearrange("(t q) d -> q t d", q=128)
            src2 = jstage[:, :].rearrange("q (t d) -> q t d", t=KT)
            nc.sync.dma_start(out=dst, in_=src2)

    nc.compile()
    return nc


def _host_inputs(x, x_noise, z, W1, b1, W2, b2, b3, b_r):
    """Build per-core input maps (all host-side transposes happen here)."""
    import ml_dtypes
    w1 = np.ascontiguousarray(
        np.concatenate([W1, W1], axis=0)).astype(ml_dtypes.bfloat16)
    w1t = np.ascontiguousarray(
        W1.T.reshape(KT, 128, C0).transpose(1, 0, 2).reshape(128, KT * C0),
        dtype=np.float32)
    w1a = np.concatenate([W1, b_r[None, :]], axis=0).astype(np.float32)
    w2 = np.ascontiguousarray(W2, dtype=np.float32)
    w2t = np.ascontiguousarray(W2.T, dtype=np.float32)
    b1c = np.ascontiguousarray(np.tile(b1, 2)[:, None], dtype=np.float32)
    b2c = np.ascontiguousarray(b2[:, None], dtype=np.float32)
    b3c = np.ascontiguousarray(b3[:, None], dtype=np.float32)

    in_maps = []
    for c in range(NCORES):
        sl = slice(c * BS, (c + 1) * BS)
        X = np.concatenate([x[sl], x_noise[sl], z[sl]], axis=0)  # [48, 1024]
        xt3 = np.ascontiguousarray(
            X.T.reshape(KT, 128, NPAIR).transpose(1, 0, 2).reshape(128, KT * NPAIR),
            dtype=np.float32)
        in_maps.append({
            "xt3": xt3, "w1": w1, "w1t": w1t, "w1a": w1a,
            "w2": w2, "w2r": w2.astype(ml_dtypes.bfloat16), "w2t": w2t,
            "b1c": b1c, "b2c": b2c, "b3c": b3c,
        })
    return in_maps


def kernel(x, x_noise, z, W1, b1, W2, b2, b3, b_r):
    global LAST_RESULT
    from concourse.bass_utils import run_bass_kernel_spmd

    x = np.asarray(x, dtype=np.float32)
    x_noise = np.asarray(x_noise, dtype=np.float32)
    z = np.asarray(z, dtype=np.float32)
    W1 = np.asarray(W1, dtype=np.float32)
    b1 = np.asarray(b1, dtype=np.float32)
    W2 = np.asarray(W2, dtype=np.float32)
    b2 = np.asarray(b2, dtype=np.float32)
    b3 = np.asarray(b3, dtype=np.float32)
    b_r = np.asarray(b_r, dtype=np.float32)

    nc = _build_nc()
    in_maps = _host_inputs(x, x_noise, z, W1, b1, W2, b2, b3, b_r)
    trace = bool(int(os.environ.get("KERNEL_TRACE", "0")))
    res = run_bass_kernel_spmd(nc, in_maps, core_ids=list(range(NCORES)),
                               trace=trace)
    LAST_RESULT = res
    results = res.results

    recover = np.concatenate([results[c]["out_r"] for c in range(NCORES)], axis=0)
    c2 = np.concatenate([results[c]["out_c2t"].T for c in range(NCORES)], axis=0)

    jacs = []
    for which in range(3):  # x, x_noise, z
        J = np.empty((B, D, D), dtype=np.float32)
        for c in range(NCORES):
            blk = np.asarray(results[c]["out_j"]).astype(np.float32)
            blk = blk.reshape(3, BS, D, D)[which]
            J[c * BS:(c + 1) * BS] = blk
        # reference's cat(dim=1)+reshape batch scrambling
        J = J.transpose(1, 0, 2).reshape(D, B * D).reshape(B, D, D)
        jacs.append(J)

    return (recover, c2, jacs[0], jacs[1], jacs[2])


# revision 25
# speedup vs baseline: 1.0653x; 1.0591x over previous
"""Trainium2 Bass kernel for nn_ALTER2Layer (dense_mlp, 8-core data parallel).

Math per batch b:
  c1 = sig(x W1^T + b1); c2 = sig(c1 W2^T + b2); c3 = sig(c2 W2 + b3)
  r  = sig(c3 W1 + b_r)
  s_i = c_i (1 - c_i)
  J[b] = W1^T D1 W2^T D2 W2 D3 W1          (D_i = diag(s_i))
Factored on device as (bf16 operands, fp32 PSUM accumulation):
  V  = D2 W2 D3 W1         [32, 1024]
  Q  = D1 W2^T V           [64, 1024]   (built twice, rows 0-63 and 64-127,
                                         by col-tiled matmuls sharing one rhs)
  J  = W1^T Q              [1024, 1024] (i-tiles alternate PE row halves so
                                         each matmul's weight load overlaps
                                         the previous matmul's stream)
The Jacobian is staged in bf16 (halves HBM write traffic); the host widens
back to fp32. All weight/input transposes and the reference's
batch-interleave reshape of the Jacobians happen host-side.

Sharding: pure data parallel, batch dim 128 -> 16 per core across 8 cores.
Each core processes 48 (input, batch) pairs: [x | x_noise | z] x 16.
"""

import os
import numpy as np

B = 128
D = 1024
C0 = 64
C1 = 32
NCORES = 8
BS = B // NCORES          # batches per core = 16
NPAIR = 3 * BS            # (input, batch) pairs per core = 48
KT = D // 128             # 8 k-tiles of 128

LAST_RESULT = None        # BassKernelResults of the most recent run (for test.py)


def _build_nc():
    import concourse.bass as bass
    import concourse.mybir as mybir
    from concourse import bacc
    from concourse.tile import TileContext, add_dep_helper
    from contextlib import ExitStack

    f32 = mybir.dt.float32
    bf16 = mybir.dt.bfloat16
    AF = mybir.ActivationFunctionType

    nc = bacc.Bacc()

    # Dummy 1x1 "absorber" matmuls read one foreign-engine-produced tile
    # each, so the following real matmuls carry few semaphore waits.
    _scratch = {"pool": None}

    def absorb(ap):
        scr = _scratch["pool"].tile([1, 1], f32, tag="ps", name="scr")
        return nc.tensor.matmul(scr[0:1, 0:1], lhsT=ap, rhs=ap,
                                start=True, stop=True)

    def order_after(inst, fence):
        if fence is not None:
            add_dep_helper(inst.ins, fence.ins, sync=False, reason="fence order")

    # ---- DRAM parameters (per-core shards / replicated weights) ----
    xt3_d = nc.dram_tensor("xt3", [128, KT * NPAIR], f32, kind="ExternalInput")
    # W1 stacked twice on the partition axis (rows 0-63 == rows 64-127)
    w1_d = nc.dram_tensor("w1", [128, D], bf16, kind="ExternalInput")
    w1t_d = nc.dram_tensor("w1t", [128, KT * C0], f32, kind="ExternalInput")
    w1a_d = nc.dram_tensor("w1a", [C0 + 1, D], f32, kind="ExternalInput")
    w2_d = nc.dram_tensor("w2", [C1, C0], f32, kind="ExternalInput")
    w2r_d = nc.dram_tensor("w2r", [C1, C0], bf16, kind="ExternalInput")
    w2t_d = nc.dram_tensor("w2t", [C0, C1], f32, kind="ExternalInput")
    b1_d = nc.dram_tensor("b1c", [128, 1], f32, kind="ExternalInput")
    b2_d = nc.dram_tensor("b2c", [C1, 1], f32, kind="ExternalInput")
    b3_d = nc.dram_tensor("b3c", [C0, 1], f32, kind="ExternalInput")

    out_r = nc.dram_tensor("out_r", [BS, D], f32, kind="ExternalOutput")
    out_c2t = nc.dram_tensor("out_c2t", [C1, BS], f32, kind="ExternalOutput")
    # 48 Jacobians stacked: rows [p*1024, (p+1)*1024) = J of pair p (bf16)
    out_j = nc.dram_tensor("out_j", [NPAIR * D, D], bf16, kind="ExternalOutput")

    with TileContext(nc) as tc, ExitStack() as stk:
        const = stk.enter_context(tc.tile_pool(name="const", bufs=1))
        enc = stk.enter_context(tc.tile_pool(name="enc", bufs=1))
        uv = stk.enter_context(tc.tile_pool(name="uv", bufs=3))
        jst = stk.enter_context(tc.tile_pool(name="jst", bufs=3))
        ps_a = stk.enter_context(tc.tile_pool(name="ps_a", bufs=2, space="PSUM"))
        ps_j = stk.enter_context(tc.tile_pool(name="ps_j", bufs=3, space="PSUM"))
        _scratch["pool"] = ps_a

        # ---- load constants ----
        xt_sb = const.tile([128, KT * NPAIR], f32)
        nc.sync.dma_start(out=xt_sb[:, :], in_=xt3_d[:, :])
        w1_sb = const.tile([128, D], bf16)
        nc.sync.dma_start(out=w1_sb[:, :], in_=w1_d[:, :])
        w1t_sb = const.tile([128, KT * C0], f32)
        nc.sync.dma_start(out=w1t_sb[:, :], in_=w1t_d[:, :])
        w1a_sb = const.tile([C0 + 1, D], f32)
        nc.sync.dma_start(out=w1a_sb[:, :], in_=w1a_d[:, :])
        w2_sb = const.tile([C1, C0], f32)
        nc.sync.dma_start(out=w2_sb[:, :], in_=w2_d[:, :])
        w2r_sb = const.tile([C1, C0], bf16)
        nc.sync.dma_start(out=w2r_sb[:, :], in_=w2r_d[:, :])
        w2t_sb = const.tile([C0, C1], f32)
        nc.sync.dma_start(out=w2t_sb[:, :], in_=w2t_d[:, :])
        b1_sb = const.tile([128, 1], f32)
        nc.sync.dma_start(out=b1_sb[:, :], in_=b1_d[:, :])
        b2_sb = const.tile([C1, 1], f32)
        nc.sync.dma_start(out=b2_sb[:, :], in_=b2_d[:, :])
        b3_sb = const.tile([C0, 1], f32)
        nc.sync.dma_start(out=b3_sb[:, :], in_=b3_d[:, :])

        fence0 = None
        for t in (xt_sb, w1_sb, w1t_sb, w1a_sb, w2_sb, w2r_sb, w2t_sb,
                  b1_sb, b2_sb, b3_sb):
            fence0 = absorb(t[0:1, 0:1])

        # ---- encode all 48 pairs at once (feature-on-partition layouts) ----
        # c1, col-tiled x2 so s1 comes out replicated on both 64-row halves
        c1_ps = ps_a.tile([128, NPAIR], f32, tag="ps", name="c1_ps")
        for cg in (0, C0):
            for k in range(KT):
                mm = nc.tensor.matmul(
                    c1_ps[cg:cg + C0, :],
                    lhsT=w1t_sb[:, k * C0:(k + 1) * C0],
                    rhs=xt_sb[:, k * NPAIR:(k + 1) * NPAIR],
                    start=(k == 0), stop=(k == KT - 1),
                    tile_position=(0, cg),
                )
                if cg == 0 and k == 0:
                    order_after(mm, fence0)
        c1t2 = enc.tile([128, NPAIR], f32)
        nc.scalar.activation(c1t2[:, :], c1_ps[:, :], AF.Sigmoid, bias=b1_sb[:, :])
        om1 = enc.tile([128, NPAIR], f32, tag="om")
        nc.scalar.activation(om1[:, :], c1t2[:, :], AF.Copy, bias=1.0, scale=-1.0)
        s1t2 = enc.tile([128, NPAIR], f32)
        nc.vector.tensor_mul(s1t2[:, :], c1t2[:, :], om1[:, :])

        # c2t[j, p] = sig(sum_c W2[j,c] c1t[c,p] + b2[j])
        c2_ps = ps_a.tile([C1, NPAIR], f32, tag="ps", name="c2_ps")
        nc.tensor.matmul(c2_ps[:, :], lhsT=w2t_sb[:, :], rhs=c1t2[:C0, :],
                         start=True, stop=True)
        c2t = enc.tile([C1, NPAIR], f32)
        nc.scalar.activation(c2t[:, :], c2_ps[:, :], AF.Sigmoid, bias=b2_sb[:, :])
        om2 = enc.tile([128, NPAIR], f32, tag="om")
        nc.scalar.activation(om2[:C1, :], c2t[:, :], AF.Copy, bias=1.0, scale=-1.0)
        s2t = enc.tile([C1, NPAIR], f32)
        nc.vector.tensor_mul(s2t[:, :], c2t[:, :], om2[:C1, :])

        # c3t[c, p] = sig(sum_j W2[j,c] c2t[j,p] + b3[c]); ones row appended
        fence_c3 = absorb(s2t[0:1, 0:1])
        c3_ps = ps_a.tile([C0, NPAIR], f32, tag="ps", name="c3_ps")
        mm = nc.tensor.matmul(c3_ps[:, :], lhsT=w2_sb[:, :], rhs=c2t[:, :],
                              start=True, stop=True)
        order_after(mm, fence_c3)
        c3ta = enc.tile([C0 + 1, NPAIR], f32)
        nc.scalar.activation(c3ta[:C0, :], c3_ps[:, :], AF.Sigmoid, bias=b3_sb[:, :])
        nc.vector.memset(c3ta[C0:C0 + 1, :], 1.0)
        om3 = enc.tile([128, NPAIR], f32, tag="om")
        nc.scalar.activation(om3[:C0, :], c3ta[:C0, :], AF.Copy, bias=1.0, scale=-1.0)
        s3t = enc.tile([C0, NPAIR], f32)
        nc.vector.tensor_mul(s3t[:, :], c3ta[:C0, :], om3[:C0, :])

        # recover (x pairs only): r = sig(c3 W1 + b_r) via W1-augmented
        fence_r = absorb(c3ta[C0:C0 + 1, 0:1])
        r_sb = enc.tile([BS, D], f32)
        for h in range(2):
            r_ps = ps_a.tile([BS, 512], f32, tag="ps", name="r_ps")
            mm = nc.tensor.matmul(
                r_ps[:, :], lhsT=c3ta[:, 0:BS],
                rhs=w1a_sb[:, h * 512:(h + 1) * 512], start=True, stop=True)
            if h == 0:
                order_after(mm, fence_r)
            nc.scalar.activation(r_sb[:, h * 512:(h + 1) * 512], r_ps[:, :],
                                 AF.Sigmoid)
        nc.sync.dma_start(out=out_r[:, :], in_=r_sb[:, :])
        nc.sync.dma_start(out=out_c2t[:, :], in_=c2t[:, 0:BS])

        # ---- per-pair Jacobian ----
        for p in range(NPAIR):
            # lhsT for V: W2^T scaled by s3 along partitions (c)
            w2t_s3 = uv.tile([C0, C1], bf16, tag="w2ts3")
            nc.vector.tensor_scalar_mul(w2t_s3[:, :], w2t_sb[:, :],
                                        s3t[:, p:p + 1])
            # V = D2 W2 D3 W1  [32, 1024]; s2 applied on the PSUM drain
            v_sb = uv.tile([C1, D], bf16, tag="v")
            for h in range(2):
                v_ps = ps_a.tile([C1, 512], f32, tag="ps", name="v_ps")
                nc.tensor.matmul(v_ps[:, :], lhsT=w2t_s3[:, :],
                                 rhs=w1_sb[0:C0, h * 512:(h + 1) * 512],
                                 start=True, stop=True)
                nc.scalar.activation(v_sb[:, h * 512:(h + 1) * 512], v_ps[:, :],
                                     AF.Copy, scale=s2t[:, p:p + 1])

            # Q = D1 W2^T V, built on both 64-row halves by col-tiled matmuls
            # sharing the same rhs stream; s1 applied on the PSUM drain
            q2_sb = uv.tile([128, D], bf16, tag="q")
            for h in range(2):
                q_ps = ps_a.tile([128, 512], f32, tag="ps", name="q_ps")
                for cg in (0, C0):
                    nc.tensor.matmul(q_ps[cg:cg + C0, :], lhsT=w2r_sb[:, :],
                                     rhs=v_sb[:, h * 512:(h + 1) * 512],
                                     start=True, stop=True,
                                     tile_position=(0, cg))
                nc.scalar.activation(q2_sb[:, h * 512:(h + 1) * 512], q_ps[:, :],
                                     AF.Copy, scale=s1t2[:, p:p + 1])

            # J = W1^T Q: i-tiles processed in (even, odd) couples whose
            # matmuls interleave across the two PE row halves, so each
            # matmul's weight load and stream overlap the other half's.
            # Drains are whole-psum [128, 1024] casts split across DVE/ACT.
            fence_j = absorb(q2_sb[0:1, 0:1])
            jstage = jst.tile([128, KT * D], bf16, tag="jst")
            for ii in range(0, KT, 2):
                pj0 = ps_j.tile([128, D], f32, tag="pj", name="pj0")
                pj1 = ps_j.tile([128, D], f32, tag="pj", name="pj1")
                for h in range(2):
                    for k, pj in ((0, pj0), (1, pj1)):
                        i = ii + k
                        rg = C0 if (i % 2) else 0
                        mm = nc.tensor.matmul(
                            pj[:, h * 512:(h + 1) * 512],
                            lhsT=w1_sb[rg:rg + C0, i * 128:(i + 1) * 128],
                            rhs=q2_sb[rg:rg + C0, h * 512:(h + 1) * 512],
                            start=True, stop=True)
                        if ii == 0 and h == 0 and k == 0:
                            order_after(mm, fence_j)
                for k, pj in ((0, pj0), (1, pj1)):
                    i = ii + k
                    dst = jstage[:, i * D:(i + 1) * D]
                    # per-pair split: 4 DVE / 4 ACT on even pairs, 3/5 on odd
                    n_dve = 4 if (p % 2 == 0) else 3
                    if i < n_dve:
                        nc.vector.tensor_copy(dst, pj[:, :])
                    else:
                        nc.scalar.copy(dst, pj[:, :])
            # dest rows p*1024 + i*128 + q are contiguous (2KB runs) per i
            dst = out_j[p * D:(p + 1) * D, :].rearrange("(t q) d -> q t d", q=128)
            src2 = jstage[:, :].rearrange("q (t d) -> q t d", t=KT)
            nc.sync.dma_start(out=dst, in_=src2)

    nc.compile()
    return nc


def _host_inputs(x, x_noise, z, W1, b1, W2, b2, b3, b_r):
    """Build per-core input maps (all host-side transposes happen here)."""
    import ml_dtypes
    w1 = np.ascontiguousarray(
        np.concatenate([W1, W1], axis=0)).astype(ml_dtypes.bfloat16)
    w1t = np.ascontiguousarray(
        W1.T.reshape(KT, 128, C0).transpose(1, 0, 2).reshape(128, KT * C0),
        dtype=np.float32)
    w1a = np.concatenate([W1, b_r[None, :]], axis=0).astype(np.float32)
    w2 = np.ascontiguousarray(W2, dtype=np.float32)
    w2t = np.ascontiguousarray(W2.T, dtype=np.float32)
    b1c = np.ascontiguousarray(np.tile(b1, 2)[:, None], dtype=np.float32)
    b2c = np.ascontiguousarray(b2[:, None], dtype=np.float32)
    b3c = np.ascontiguousarray(b3[:, None], dtype=np.float32)

    in_maps = []
    for c in range(NCORES):
        sl = slice(c * BS, (c + 1) * BS)
        X = np.concatenate([x[sl], x_noise[sl], z[sl]], axis=0)  # [48, 1024]
        xt3 = np.ascontiguousarray(
            X.T.reshape(KT, 128, NPAIR).transpose(1, 0, 2).reshape(128, KT * NPAIR),
            dtype=np.float32)
        in_maps.append({
            "xt3": xt3, "w1": w1, "w1t": w1t, "w1a": w1a,
            "w2": w2, "w2r": w2.astype(ml_dtypes.bfloat16), "w2t": w2t,
            "b1c": b1c, "b2c": b2c, "b3c": b3c,
        })
    return in_maps


def kernel(x, x_noise, z, W1, b1, W2, b2, b3, b_r):
    global LAST_RESULT
    from concourse.bass_utils import run_bass_kernel_spmd

    x = np.asarray(x, dtype=np.float32)
    x_noise = np.asarray(x_noise, dtype=np.float32)
    z = np.asarray(z, dtype=np.float32)
    W1 = np.asarray(W1, dtype=np.float32)
    b1 = np.asarray(b1, dtype=np.float32)
    W2 = np.asarray(W2, dtype=np.float32)
    b2 = np.asarray(b2, dtype=np.float32)
    b3 = np.asarray(b3, dtype=np.float32)
    b_r = np.asarray(b_r, dtype=np.float32)

    nc = _build_nc()
    in_maps = _host_inputs(x, x_noise, z, W1, b1, W2, b2, b3, b_r)
    trace = bool(int(os.environ.get("KERNEL_TRACE", "0")))
    res = run_bass_kernel_spmd(nc, in_maps, core_ids=list(range(NCORES)),
                               trace=trace)
    LAST_RESULT = res
    results = res.results

    recover = np.concatenate([results[c]["out_r"] for c in range(NCORES)], axis=0)
    c2 = np.concatenate([results[c]["out_c2t"].T for c in range(NCORES)], axis=0)

    jacs = []
    for which in range(3):  # x, x_noise, z
        J = np.empty((B, D, D), dtype=np.float32)
        for c in range(NCORES):
            blk = np.asarray(results[c]["out_j"]).astype(np.float32)
            blk = blk.reshape(3, BS, D, D)[which]
            J[c * BS:(c + 1) * BS] = blk
        # reference's cat(dim=1)+reshape batch scrambling
        J = J.transpose(1, 0, 2).reshape(D, B * D).reshape(B, D, D)
        jacs.append(J)

    return (recover, c2, jacs[0], jacs[1], jacs[2])


# revision 27
# speedup vs baseline: 1.1839x; 1.1113x over previous
"""Trainium2 Bass kernel for nn_ALTER2Layer (dense_mlp, 8-core data parallel).

Math per batch b:
  c1 = sig(x W1^T + b1); c2 = sig(c1 W2^T + b2); c3 = sig(c2 W2 + b3)
  r  = sig(c3 W1 + b_r)
  s_i = c_i (1 - c_i)
  J[b] = W1^T D1 W2^T D2 W2 D3 W1          (D_i = diag(s_i))
Factored on device as (bf16 operands, fp32 PSUM accumulation):
  V  = D2 W2 D3 W1         [32, 1024]
  Q  = D1 W2^T V           [64, 1024]   (built twice, rows 0-63 and 64-127,
                                         by col-tiled matmuls sharing one rhs)
  J  = W1^T Q              [1024, 1024] (i-tiles alternate PE row halves so
                                         each matmul's weight load overlaps
                                         the previous matmul's stream)
The Jacobian is staged in bf16 (halves HBM write traffic); the host widens
back to fp32. All weight/input transposes and the reference's
batch-interleave reshape of the Jacobians happen host-side.

Sharding: pure data parallel, batch dim 128 -> 16 per core across 8 cores.
Each core processes 48 (input, batch) pairs: [x | x_noise | z] x 16.
"""

import os
import numpy as np

B = 128
D = 1024
C0 = 64
C1 = 32
NCORES = 8
BS = B // NCORES          # batches per core = 16
NPAIR = 3 * BS            # (input, batch) pairs per core = 48
KT = D // 128             # 8 k-tiles of 128

LAST_RESULT = None        # BassKernelResults of the most recent run (for test.py)


def _build_nc():
    import concourse.bass as bass
    import concourse.mybir as mybir
    from concourse import bacc
    from concourse.tile import TileContext, add_dep_helper
    from contextlib import ExitStack

    f32 = mybir.dt.float32
    bf16 = mybir.dt.bfloat16
    AF = mybir.ActivationFunctionType

    nc = bacc.Bacc()

    # Dummy 1x1 "absorber" matmuls read one foreign-engine-produced tile
    # each, so the following real matmuls carry few semaphore waits.
    _scratch = {"pool": None}

    def absorb(ap):
        scr = _scratch["pool"].tile([1, 1], f32, tag="ps", name="scr")
        return nc.tensor.matmul(scr[0:1, 0:1], lhsT=ap, rhs=ap,
                                start=True, stop=True)

    def order_after(inst, fence):
        if fence is not None:
            add_dep_helper(inst.ins, fence.ins, sync=False, reason="fence order")

    # ---- DRAM parameters (per-core shards / replicated weights) ----
    xt3_d = nc.dram_tensor("xt3", [128, KT * NPAIR], f32, kind="ExternalInput")
    # W1 stacked twice on the partition axis (rows 0-63 == rows 64-127)
    w1_d = nc.dram_tensor("w1", [128, D], bf16, kind="ExternalInput")
    w1t_d = nc.dram_tensor("w1t", [128, KT * C0], f32, kind="ExternalInput")
    w1a_d = nc.dram_tensor("w1a", [C0 + 1, D], f32, kind="ExternalInput")
    w2_d = nc.dram_tensor("w2", [C1, C0], f32, kind="ExternalInput")
    w2r_d = nc.dram_tensor("w2r", [C1, C0], bf16, kind="ExternalInput")
    w2t_d = nc.dram_tensor("w2t", [C0, C1], f32, kind="ExternalInput")
    b1_d = nc.dram_tensor("b1c", [128, 1], f32, kind="ExternalInput")
    b2_d = nc.dram_tensor("b2c", [C1, 1], f32, kind="ExternalInput")
    b3_d = nc.dram_tensor("b3c", [C0, 1], f32, kind="ExternalInput")

    out_r = nc.dram_tensor("out_r", [BS, D], f32, kind="ExternalOutput")
    out_c2t = nc.dram_tensor("out_c2t", [C1, BS], f32, kind="ExternalOutput")
    # 48 Jacobians stacked: rows [p*1024, (p+1)*1024) = J of pair p (bf16)
    out_j = nc.dram_tensor("out_j", [NPAIR * D, D], bf16, kind="ExternalOutput")

    with TileContext(nc) as tc, ExitStack() as stk:
        const = stk.enter_context(tc.tile_pool(name="const", bufs=1))
        enc = stk.enter_context(tc.tile_pool(name="enc", bufs=1))
        uv = stk.enter_context(tc.tile_pool(name="uv", bufs=3))
        jst = stk.enter_context(tc.tile_pool(name="jst", bufs=3))
        ps_a = stk.enter_context(tc.tile_pool(name="ps_a", bufs=2, space="PSUM"))
        ps_j = stk.enter_context(tc.tile_pool(name="ps_j", bufs=3, space="PSUM"))
        _scratch["pool"] = ps_a

        # ---- load constants ----
        xt_sb = const.tile([128, KT * NPAIR], f32)
        nc.sync.dma_start(out=xt_sb[:, :], in_=xt3_d[:, :])
        w1_sb = const.tile([128, D], bf16)
        nc.sync.dma_start(out=w1_sb[:, :], in_=w1_d[:, :])
        w1t_sb = const.tile([128, KT * C0], f32)
        nc.sync.dma_start(out=w1t_sb[:, :], in_=w1t_d[:, :])
        w1a_sb = const.tile([C0 + 1, D], f32)
        nc.sync.dma_start(out=w1a_sb[:, :], in_=w1a_d[:, :])
        w2_sb = const.tile([C1, C0], f32)
        nc.sync.dma_start(out=w2_sb[:, :], in_=w2_d[:, :])
        w2r_sb = const.tile([C1, C0], bf16)
        nc.sync.dma_start(out=w2r_sb[:, :], in_=w2r_d[:, :])
        w2t_sb = const.tile([C0, C1], f32)
        nc.sync.dma_start(out=w2t_sb[:, :], in_=w2t_d[:, :])
        b1_sb = const.tile([128, 1], f32)
        nc.sync.dma_start(out=b1_sb[:, :], in_=b1_d[:, :])
        b2_sb = const.tile([C1, 1], f32)
        nc.sync.dma_start(out=b2_sb[:, :], in_=b2_d[:, :])
        b3_sb = const.tile([C0, 1], f32)
        nc.sync.dma_start(out=b3_sb[:, :], in_=b3_d[:, :])

        fence0 = None
        for t in (xt_sb, w1_sb, w1t_sb, w1a_sb, w2_sb, w2r_sb, w2t_sb,
                  b1_sb, b2_sb, b3_sb):
            fence0 = absorb(t[0:1, 0:1])

        # ---- encode all 48 pairs at once (feature-on-partition layouts) ----
        # c1, col-tiled x2 so s1 comes out replicated on both 64-row halves
        c1_ps = ps_a.tile([128, NPAIR], f32, tag="ps", name="c1_ps")
        for cg in (0, C0):
            for k in range(KT):
                mm = nc.tensor.matmul(
                    c1_ps[cg:cg + C0, :],
                    lhsT=w1t_sb[:, k * C0:(k + 1) * C0],
                    rhs=xt_sb[:, k * NPAIR:(k + 1) * NPAIR],
                    start=(k == 0), stop=(k == KT - 1),
                    tile_position=(0, cg),
                )
                if cg == 0 and k == 0:
                    order_after(mm, fence0)
        c1t2 = enc.tile([128, NPAIR], f32)
        nc.scalar.activation(c1t2[:, :], c1_ps[:, :], AF.Sigmoid, bias=b1_sb[:, :])
        om1 = enc.tile([128, NPAIR], f32, tag="om")
        nc.scalar.activation(om1[:, :], c1t2[:, :], AF.Copy, bias=1.0, scale=-1.0)
        s1t2 = enc.tile([128, NPAIR], f32)
        nc.vector.tensor_mul(s1t2[:, :], c1t2[:, :], om1[:, :])

        # c2t[j, p] = sig(sum_c W2[j,c] c1t[c,p] + b2[j])
        c2_ps = ps_a.tile([C1, NPAIR], f32, tag="ps", name="c2_ps")
        nc.tensor.matmul(c2_ps[:, :], lhsT=w2t_sb[:, :], rhs=c1t2[:C0, :],
                         start=True, stop=True)
        c2t = enc.tile([C1, NPAIR], f32)
        nc.scalar.activation(c2t[:, :], c2_ps[:, :], AF.Sigmoid, bias=b2_sb[:, :])
        om2 = enc.tile([128, NPAIR], f32, tag="om")
        nc.scalar.activation(om2[:C1, :], c2t[:, :], AF.Copy, bias=1.0, scale=-1.0)
        s2t = enc.tile([C1, NPAIR], f32)
        nc.vector.tensor_mul(s2t[:, :], c2t[:, :], om2[:C1, :])

        # c3t[c, p] = sig(sum_j W2[j,c] c2t[j,p] + b3[c]); ones row appended
        fence_c3 = absorb(s2t[0:1, 0:1])
        c3_ps = ps_a.tile([C0, NPAIR], f32, tag="ps", name="c3_ps")
        mm = nc.tensor.matmul(c3_ps[:, :], lhsT=w2_sb[:, :], rhs=c2t[:, :],
                              start=True, stop=True)
        order_after(mm, fence_c3)
        c3ta = enc.tile([C0 + 1, NPAIR], f32)
        nc.scalar.activation(c3ta[:C0, :], c3_ps[:, :], AF.Sigmoid, bias=b3_sb[:, :])
        nc.vector.memset(c3ta[C0:C0 + 1, :], 1.0)
        om3 = enc.tile([128, NPAIR], f32, tag="om")
        nc.scalar.activation(om3[:C0, :], c3ta[:C0, :], AF.Copy, bias=1.0, scale=-1.0)
        s3t = enc.tile([C0, NPAIR], f32)
        nc.vector.tensor_mul(s3t[:, :], c3ta[:C0, :], om3[:C0, :])

        # recover (x pairs only): r = sig(c3 W1 + b_r) via W1-augmented
        fence_r = absorb(c3ta[C0:C0 + 1, 0:1])
        r_sb = enc.tile([BS, D], f32)
        for h in range(2):
            r_ps = ps_a.tile([BS, 512], f32, tag="ps", name="r_ps")
            mm = nc.tensor.matmul(
                r_ps[:, :], lhsT=c3ta[:, 0:BS],
                rhs=w1a_sb[:, h * 512:(h + 1) * 512], start=True, stop=True)
            if h == 0:
                order_after(mm, fence_r)
            nc.scalar.activation(r_sb[:, h * 512:(h + 1) * 512], r_ps[:, :],
                                 AF.Sigmoid)
        nc.sync.dma_start(out=out_r[:, :], in_=r_sb[:, :])
        nc.sync.dma_start(out=out_c2t[:, :], in_=c2t[:, 0:BS])

        # ---- per-pair Jacobian ----
        # All matmuls stream N=1024 (bf16 moving operand limit); V/Q/J psums
        # share the 2-bank ps_j pool. J i-tiles run in (even, odd) couples on
        # alternating PE row halves so weight loads and streams overlap.
        for p in range(NPAIR):
            w2t_s3 = uv.tile([C0, C1], bf16, tag="w2ts3")
            nc.vector.tensor_scalar_mul(w2t_s3[:, :], w2t_sb[:, :],
                                        s3t[:, p:p + 1])
            # V = D2 W2 D3 W1  [32, 1024]; s2 applied on the PSUM drain
            v_sb = uv.tile([C1, D], bf16, tag="v")
            v_ps = ps_j.tile([C1, D], f32, tag="pj", name="v_ps")
            for h in range(2):
                nc.tensor.matmul(v_ps[:, h * 512:(h + 1) * 512],
                                 lhsT=w2t_s3[:, :],
                                 rhs=w1_sb[0:C0, h * 512:(h + 1) * 512],
                                 start=True, stop=True)
            nc.scalar.activation(v_sb[:, :], v_ps[:, :], AF.Copy,
                                 scale=s2t[:, p:p + 1])

            # Q = D1 W2^T V on both 64-row halves (col-tiled, shared rhs)
            q2_sb = uv.tile([128, D], bf16, tag="q")
            q_ps = ps_j.tile([128, D], f32, tag="pj", name="q_ps")
            for h in range(2):
                for cg in (0, C0):
                    nc.tensor.matmul(q_ps[cg:cg + C0, h * 512:(h + 1) * 512],
                                     lhsT=w2r_sb[:, :],
                                     rhs=v_sb[:, h * 512:(h + 1) * 512],
                                     start=True, stop=True,
                                     tile_position=(0, cg))
            nc.scalar.activation(q2_sb[:, :], q_ps[:, :], AF.Copy,
                                 scale=s1t2[:, p:p + 1])

            # J = W1^T Q: one N=1024 matmul per i-tile, couples alternate
            # row halves; drains split across DVE/ACT by a rotating pattern
            fence_j = absorb(q2_sb[0:1, 0:1])
            jstage = jst.tile([128, KT * D], bf16, tag="jst")
            for ii in range(0, KT, 2):
                pjs = [(ii, ps_j.tile([128, D], f32, tag="pj", name="pj0")),
                       (ii + 1, ps_j.tile([128, D], f32, tag="pj", name="pj1"))]
                for h in range(2):
                    for i, pj in pjs:
                        rg = C0 if (i % 2) else 0
                        mm = nc.tensor.matmul(
                            pj[:, h * 512:(h + 1) * 512],
                            lhsT=w1_sb[rg:rg + C0, i * 128:(i + 1) * 128],
                            rhs=q2_sb[rg:rg + C0, h * 512:(h + 1) * 512],
                            start=True, stop=True)
                        if ii == 0 and h == 0 and i == ii:
                            order_after(mm, fence_j)
                for i, pj in pjs:
                    dst = jstage[:, i * D:(i + 1) * D]
                    # 4 DVE / 4 ACT with the odd pair favoring ACT
                    if (i + p) % 2 == 0:
                        nc.vector.tensor_copy(dst, pj[:, :])
                    else:
                        nc.scalar.copy(dst, pj[:, :])
            # dest rows p*1024 + i*128 + q are contiguous (2KB runs) per i
            dst = out_j[p * D:(p + 1) * D, :].rearrange("(t q) d -> q t d", q=128)
            src2 = jstage[:, :].rearrange("q (t d) -> q t d", t=KT)
            nc.sync.dma_start(out=dst, in_=src2)

    nc.compile()
    return nc


def _host_inputs(x, x_noise, z, W1, b1, W2, b2, b3, b_r):
    """Build per-core input maps (all host-side transposes happen here)."""
    import ml_dtypes
    w1 = np.ascontiguousarray(
        np.concatenate([W1, W1], axis=0)).astype(ml_dtypes.bfloat16)
    w1t = np.ascontiguousarray(
        W1.T.reshape(KT, 128, C0).transpose(1, 0, 2).reshape(128, KT * C0),
        dtype=np.float32)
    w1a = np.concatenate([W1, b_r[None, :]], axis=0).astype(np.float32)
    w2 = np.ascontiguousarray(W2, dtype=np.float32)
    w2t = np.ascontiguousarray(W2.T, dtype=np.float32)
    b1c = np.ascontiguousarray(np.tile(b1, 2)[:, None], dtype=np.float32)
    b2c = np.ascontiguousarray(b2[:, None], dtype=np.float32)
    b3c = np.ascontiguousarray(b3[:, None], dtype=np.float32)

    in_maps = []
    for c in range(NCORES):
        sl = slice(c * BS, (c + 1) * BS)
        X = np.concatenate([x[sl], x_noise[sl], z[sl]], axis=0)  # [48, 1024]
        xt3 = np.ascontiguousarray(
            X.T.reshape(KT, 128, NPAIR).transpose(1, 0, 2).reshape(128, KT * NPAIR),
            dtype=np.float32)
        in_maps.append({
            "xt3": xt3, "w1": w1, "w1t": w1t, "w1a": w1a,
            "w2": w2, "w2r": w2.astype(ml_dtypes.bfloat16), "w2t": w2t,
            "b1c": b1c, "b2c": b2c, "b3c": b3c,
        })
    return in_maps


def kernel(x, x_noise, z, W1, b1, W2, b2, b3, b_r):
    global LAST_RESULT
    from concourse.bass_utils import run_bass_kernel_spmd

    x = np.asarray(x, dtype=np.float32)
    x_noise = np.asarray(x_noise, dtype=np.float32)
    z = np.asarray(z, dtype=np.float32)
    W1 = np.asarray(W1, dtype=np.float32)
    b1 = np.asarray(b1, dtype=np.float32)
    W2 = np.asarray(W2, dtype=np.float32)
    b2 = np.asarray(b2, dtype=np.float32)
    b3 = np.asarray(b3, dtype=np.float32)
    b_r = np.asarray(b_r, dtype=np.float32)

    nc = _build_nc()
    in_maps = _host_inputs(x, x_noise, z, W1, b1, W2, b2, b3, b_r)
    trace = bool(int(os.environ.get("KERNEL_TRACE", "0")))
    res = run_bass_kernel_spmd(nc, in_maps, core_ids=list(range(NCORES)),
                               trace=trace)
    LAST_RESULT = res
    results = res.results

    recover = np.concatenate([results[c]["out_r"] for c in range(NCORES)], axis=0)
    c2 = np.concatenate([results[c]["out_c2t"].T for c in range(NCORES)], axis=0)

    jacs = []
    for which in range(3):  # x, x_noise, z
        J = np.empty((B, D, D), dtype=np.float32)
        for c in range(NCORES):
            blk = np.asarray(results[c]["out_j"]).astype(np.float32)
            blk = blk.reshape(3, BS, D, D)[which]
            J[c * BS:(c + 1) * BS] = blk
        # reference's cat(dim=1)+reshape batch scrambling
        J = J.transpose(1, 0, 2).reshape(D, B * D).reshape(B, D, D)
        jacs.append(J)

    return (recover, c2, jacs[0], jacs[1], jacs[2])


# revision 28
# speedup vs baseline: 1.4374x; 1.2141x over previous
"""Trainium2 Bass kernel for nn_ALTER2Layer (dense_mlp, 8-core data parallel).

Math per batch b:
  c1 = sig(x W1^T + b1); c2 = sig(c1 W2^T + b2); c3 = sig(c2 W2 + b3)
  r  = sig(c3 W1 + b_r)
  s_i = c_i (1 - c_i)
  J[b] = W1^T D1 W2^T D2 W2 D3 W1          (D_i = diag(s_i))
Factored on device as (bf16 operands, fp32 PSUM accumulation):
  V  = D2 W2 D3 W1         [32, 1024]
  Q  = D1 W2^T V           [64, 1024]   (built twice, rows 0-63 and 64-127,
                                         by col-tiled matmuls sharing one rhs)
  J  = W1^T Q              [1024, 1024] (i-tiles alternate PE row halves so
                                         each matmul's weight load overlaps
                                         the previous matmul's stream)
The Jacobian is staged in bf16 (halves HBM write traffic); the host widens
back to fp32. All weight/input transposes and the reference's
batch-interleave reshape of the Jacobians happen host-side.

Sharding: pure data parallel, batch dim 128 -> 16 per core across 8 cores.
Each core processes 48 (input, batch) pairs: [x | x_noise | z] x 16.
"""

import os
import numpy as np

B = 128
D = 1024
C0 = 64
C1 = 32
NCORES = 8
BS = B // NCORES          # batches per core = 16
NPAIR = 3 * BS            # (input, batch) pairs per core = 48
KT = D // 128             # 8 k-tiles of 128

LAST_RESULT = None        # BassKernelResults of the most recent run (for test.py)


def _build_nc():
    import concourse.bass as bass
    import concourse.mybir as mybir
    from concourse import bacc
    from concourse.tile import TileContext, add_dep_helper
    from contextlib import ExitStack

    f32 = mybir.dt.float32
    bf16 = mybir.dt.bfloat16
    AF = mybir.ActivationFunctionType

    nc = bacc.Bacc()

    # Dummy 1x1 "absorber" matmuls read one foreign-engine-produced tile
    # each, so the following real matmuls carry few semaphore waits.
    _scratch = {"pool": None}

    def absorb(ap):
        scr = _scratch["pool"].tile([1, 1], f32, tag="ps", name="scr")
        return nc.tensor.matmul(scr[0:1, 0:1], lhsT=ap, rhs=ap,
                                start=True, stop=True)

    def order_after(inst, fence):
        if fence is not None:
            add_dep_helper(inst.ins, fence.ins, sync=False, reason="fence order")

    # ---- DRAM parameters (per-core shards / replicated weights) ----
    xt3_d = nc.dram_tensor("xt3", [128, KT * NPAIR], f32, kind="ExternalInput")
    # W1 stacked twice on the partition axis (rows 0-63 == rows 64-127)
    w1_d = nc.dram_tensor("w1", [128, D], bf16, kind="ExternalInput")
    w1t_d = nc.dram_tensor("w1t", [128, KT * C0], f32, kind="ExternalInput")
    w1a_d = nc.dram_tensor("w1a", [C0 + 1, D], f32, kind="ExternalInput")
    w2_d = nc.dram_tensor("w2", [C1, C0], f32, kind="ExternalInput")
    w2r_d = nc.dram_tensor("w2r", [C1, C0], bf16, kind="ExternalInput")
    w2t_d = nc.dram_tensor("w2t", [C0, C1], f32, kind="ExternalInput")
    b1_d = nc.dram_tensor("b1c", [128, 1], f32, kind="ExternalInput")
    b2_d = nc.dram_tensor("b2c", [C1, 1], f32, kind="ExternalInput")
    b3_d = nc.dram_tensor("b3c", [C0, 1], f32, kind="ExternalInput")

    out_r = nc.dram_tensor("out_r", [BS, D], f32, kind="ExternalOutput")
    out_c2t = nc.dram_tensor("out_c2t", [C1, BS], f32, kind="ExternalOutput")
    # 48 Jacobians stacked: rows [p*1024, (p+1)*1024) = J of pair p (bf16)
    out_j = nc.dram_tensor("out_j", [NPAIR * D, D], bf16, kind="ExternalOutput")

    with TileContext(nc) as tc, ExitStack() as stk:
        const = stk.enter_context(tc.tile_pool(name="const", bufs=1))
        enc = stk.enter_context(tc.tile_pool(name="enc", bufs=1))
        uv = stk.enter_context(tc.tile_pool(name="uv", bufs=3))
        jst = stk.enter_context(tc.tile_pool(name="jst", bufs=3))
        ps_a = stk.enter_context(tc.tile_pool(name="ps_a", bufs=2, space="PSUM"))
        ps_j = stk.enter_context(tc.tile_pool(name="ps_j", bufs=3, space="PSUM"))
        _scratch["pool"] = ps_a

        # ---- load constants ----
        xt_sb = const.tile([128, KT * NPAIR], f32)
        nc.sync.dma_start(out=xt_sb[:, :], in_=xt3_d[:, :])
        w1_sb = const.tile([128, D], bf16)
        nc.sync.dma_start(out=w1_sb[:, :], in_=w1_d[:, :])
        w1t_sb = const.tile([128, KT * C0], f32)
        nc.sync.dma_start(out=w1t_sb[:, :], in_=w1t_d[:, :])
        w1a_sb = const.tile([C0 + 1, D], f32)
        nc.sync.dma_start(out=w1a_sb[:, :], in_=w1a_d[:, :])
        w2_sb = const.tile([C1, C0], f32)
        nc.sync.dma_start(out=w2_sb[:, :], in_=w2_d[:, :])
        w2r_sb = const.tile([C1, C0], bf16)
        nc.sync.dma_start(out=w2r_sb[:, :], in_=w2r_d[:, :])
        w2t_sb = const.tile([C0, C1], f32)
        nc.sync.dma_start(out=w2t_sb[:, :], in_=w2t_d[:, :])
        b1_sb = const.tile([128, 1], f32)
        nc.sync.dma_start(out=b1_sb[:, :], in_=b1_d[:, :])
        b2_sb = const.tile([C1, 1], f32)
        nc.sync.dma_start(out=b2_sb[:, :], in_=b2_d[:, :])
        b3_sb = const.tile([C0, 1], f32)
        nc.sync.dma_start(out=b3_sb[:, :], in_=b3_d[:, :])

        fence0 = None
        for t in (xt_sb, w1_sb, w1t_sb, w1a_sb, w2_sb, w2r_sb, w2t_sb,
                  b1_sb, b2_sb, b3_sb):
            fence0 = absorb(t[0:1, 0:1])

        # ---- encode all 48 pairs at once (feature-on-partition layouts) ----
        # c1, col-tiled x2 so s1 comes out replicated on both 64-row halves
        c1_ps = ps_a.tile([128, NPAIR], f32, tag="ps", name="c1_ps")
        for cg in (0, C0):
            for k in range(KT):
                mm = nc.tensor.matmul(
                    c1_ps[cg:cg + C0, :],
                    lhsT=w1t_sb[:, k * C0:(k + 1) * C0],
                    rhs=xt_sb[:, k * NPAIR:(k + 1) * NPAIR],
                    start=(k == 0), stop=(k == KT - 1),
                    tile_position=(0, cg),
                )
                if cg == 0 and k == 0:
                    order_after(mm, fence0)
        c1t2 = enc.tile([128, NPAIR], f32)
        nc.scalar.activation(c1t2[:, :], c1_ps[:, :], AF.Sigmoid, bias=b1_sb[:, :])
        om1 = enc.tile([128, NPAIR], f32, tag="om")
        nc.scalar.activation(om1[:, :], c1t2[:, :], AF.Copy, bias=1.0, scale=-1.0)
        s1t2 = enc.tile([128, NPAIR], f32)
        nc.vector.tensor_mul(s1t2[:, :], c1t2[:, :], om1[:, :])

        # c2t[j, p] = sig(sum_c W2[j,c] c1t[c,p] + b2[j])
        c2_ps = ps_a.tile([C1, NPAIR], f32, tag="ps", name="c2_ps")
        nc.tensor.matmul(c2_ps[:, :], lhsT=w2t_sb[:, :], rhs=c1t2[:C0, :],
                         start=True, stop=True)
        c2t = enc.tile([C1, NPAIR], f32)
        nc.scalar.activation(c2t[:, :], c2_ps[:, :], AF.Sigmoid, bias=b2_sb[:, :])
        om2 = enc.tile([128, NPAIR], f32, tag="om")
        nc.scalar.activation(om2[:C1, :], c2t[:, :], AF.Copy, bias=1.0, scale=-1.0)
        s2t = enc.tile([C1, NPAIR], f32)
        nc.vector.tensor_mul(s2t[:, :], c2t[:, :], om2[:C1, :])

        # c3t[c, p] = sig(sum_j W2[j,c] c2t[j,p] + b3[c]); ones row appended
        fence_c3 = absorb(s2t[0:1, 0:1])
        c3_ps = ps_a.tile([C0, NPAIR], f32, tag="ps", name="c3_ps")
        mm = nc.tensor.matmul(c3_ps[:, :], lhsT=w2_sb[:, :], rhs=c2t[:, :],
                              start=True, stop=True)
        order_after(mm, fence_c3)
        c3ta = enc.tile([C0 + 1, NPAIR], f32)
        nc.scalar.activation(c3ta[:C0, :], c3_ps[:, :], AF.Sigmoid, bias=b3_sb[:, :])
        nc.vector.memset(c3ta[C0:C0 + 1, :], 1.0)
        om3 = enc.tile([128, NPAIR], f32, tag="om")
        nc.scalar.activation(om3[:C0, :], c3ta[:C0, :], AF.Copy, bias=1.0, scale=-1.0)
        s3t = enc.tile([C0, NPAIR], f32)
        nc.vector.tensor_mul(s3t[:, :], c3ta[:C0, :], om3[:C0, :])

        # recover (x pairs only): r = sig(c3 W1 + b_r) via W1-augmented
        fence_r = absorb(c3ta[C0:C0 + 1, 0:1])
        r_sb = enc.tile([BS, D], f32)
        for h in range(2):
            r_ps = ps_a.tile([BS, 512], f32, tag="ps", name="r_ps")
            mm = nc.tensor.matmul(
                r_ps[:, :], lhsT=c3ta[:, 0:BS],
                rhs=w1a_sb[:, h * 512:(h + 1) * 512], start=True, stop=True)
            if h == 0:
                order_after(mm, fence_r)
            nc.scalar.activation(r_sb[:, h * 512:(h + 1) * 512], r_ps[:, :],
                                 AF.Sigmoid)
        nc.sync.dma_start(out=out_r[:, :], in_=r_sb[:, :])
        nc.sync.dma_start(out=out_c2t[:, :], in_=c2t[:, 0:BS])

        # w2t * s3 for every pair, computed up front (off the critical path)
        w2ts3_all = enc.tile([C0, C1 * NPAIR], bf16)
        for p in range(NPAIR):
            nc.vector.tensor_scalar_mul(w2ts3_all[:, p * C1:(p + 1) * C1],
                                        w2t_sb[:, :], s3t[:, p:p + 1])

        # ---- per-pair Jacobian ----
        # All matmuls stream N=1024 (bf16 moving operand limit); V/Q/J psums
        # share the 2-bank ps_j pool. J i-tiles run in (even, odd) couples on
        # alternating PE row halves so weight loads and streams overlap.
        for p in range(NPAIR):
            w2t_s3 = w2ts3_all[:, p * C1:(p + 1) * C1]
            # V = D2 W2 D3 W1  [32, 1024]; s2 applied on the PSUM drain (ACT)
            v_sb = uv.tile([C1, D], bf16, tag="v")
            for h in range(2):
                v_ps = ps_a.tile([C1, 512], f32, tag="ps", name="v_ps")
                nc.tensor.matmul(v_ps[:, :], lhsT=w2t_s3,
                                 rhs=w1_sb[0:C0, h * 512:(h + 1) * 512],
                                 start=True, stop=True)
                nc.scalar.activation(v_sb[:, h * 512:(h + 1) * 512], v_ps[:, :],
                                     AF.Copy, scale=s2t[:, p:p + 1])

            # Q = D1 W2^T V on both 64-row halves (col-tiled, shared rhs);
            # s1 applied on the PSUM drain (DVE)
            q2_sb = uv.tile([128, D], bf16, tag="q")
            for h in range(2):
                q_ps = ps_a.tile([128, 512], f32, tag="ps", name="q_ps")
                for cg in (0, C0):
                    nc.tensor.matmul(q_ps[cg:cg + C0, :],
                                     lhsT=w2r_sb[:, :],
                                     rhs=v_sb[:, h * 512:(h + 1) * 512],
                                     start=True, stop=True,
                                     tile_position=(0, cg))
                nc.vector.tensor_scalar_mul(q2_sb[:, h * 512:(h + 1) * 512],
                                            q_ps[:, :], s1t2[:, p:p + 1])

            # J = W1^T Q: one N=1024 matmul per i-tile, couples alternate
            # row halves; drains split across DVE/ACT by a rotating pattern
            fence_j = absorb(q2_sb[0:1, 0:1])
            jstage = jst.tile([128, KT * D], bf16, tag="jst")
            for ii in range(0, KT, 2):
                pjs = [(ii, ps_j.tile([128, D], f32, tag="pj", name="pj0")),
                       (ii + 1, ps_j.tile([128, D], f32, tag="pj", name="pj1"))]
                for h in range(2):
                    for i, pj in pjs:
                        rg = C0 if (i % 2) else 0
                        mm = nc.tensor.matmul(
                            pj[:, h * 512:(h + 1) * 512],
                            lhsT=w1_sb[rg:rg + C0, i * 128:(i + 1) * 128],
                            rhs=q2_sb[rg:rg + C0, h * 512:(h + 1) * 512],
                            start=True, stop=True)
                        if ii == 0 and h == 0 and i == ii:
                            order_after(mm, fence_j)
                n_dve = 3 if (p % 2 == 0) else 4
                for i, pj in pjs:
                    dst = jstage[:, i * D:(i + 1) * D]
                    if i < n_dve:
                        nc.vector.tensor_copy(dst, pj[:, :])
                    else:
                        nc.scalar.copy(dst, pj[:, :])
            # dest rows p*1024 + i*128 + q are contiguous (2KB runs) per i
            dst = out_j[p * D:(p + 1) * D, :].rearrange("(t q) d -> q t d", q=128)
            src2 = jstage[:, :].rearrange("q (t d) -> q t d", t=KT)
            nc.sync.dma_start(out=dst, in_=src2)

    nc.compile()
    return nc


def _host_inputs(x, x_noise, z, W1, b1, W2, b2, b3, b_r):
    """Build per-core input maps (all host-side transposes happen here)."""
    import ml_dtypes
    w1 = np.ascontiguousarray(
        np.concatenate([W1, W1], axis=0)).astype(ml_dtypes.bfloat16)
    w1t = np.ascontiguousarray(
        W1.T.reshape(KT, 128, C0).transpose(1, 0, 2).reshape(128, KT * C0),
        dtype=np.float32)
    w1a = np.concatenate([W1, b_r[None, :]], axis=0).astype(np.float32)
    w2 = np.ascontiguousarray(W2, dtype=np.float32)
    w2t = np.ascontiguousarray(W2.T, dtype=np.float32)
    b1c = np.ascontiguousarray(np.tile(b1, 2)[:, None], dtype=np.float32)
    b2c = np.ascontiguousarray(b2[:, None], dtype=np.float32)
    b3c = np.ascontiguousarray(b3[:, None], dtype=np.float32)

    in_maps = []
    for c in range(NCORES):
        sl = slice(c * BS, (c + 1) * BS)
        X = np.concatenate([x[sl], x_noise[sl], z[sl]], axis=0)  # [48, 1024]
        xt3 = np.ascontiguousarray(
            X.T.reshape(KT, 128, NPAIR).transpose(1, 0, 2).reshape(128, KT * NPAIR),
            dtype=np.float32)
        in_maps.append({
            "xt3": xt3, "w1": w1, "w1t": w1t, "w1a": w1a,
            "w2": w2, "w2r": w2.astype(ml_dtypes.bfloat16), "w2t": w2t,
            "b1c": b1c, "b2c": b2c, "b3c": b3c,
        })
    return in_maps


def kernel(x, x_noise, z, W1, b1, W2, b2, b3, b_r):
    global LAST_RESULT
    from concourse.bass_utils import run_bass_kernel_spmd

    x = np.asarray(x, dtype=np.float32)
    x_noise = np.asarray(x_noise, dtype=np.float32)
    z = np.asarray(z, dtype=np.float32)
    W1 = np.asarray(W1, dtype=np.float32)
    b1 = np.asarray(b1, dtype=np.float32)
    W2 = np.asarray(W2, dtype=np.float32)
    b2 = np.asarray(b2, dtype=np.float32)
    b3 = np.asarray(b3, dtype=np.float32)
    b_r = np.asarray(b_r, dtype=np.float32)

    nc = _build_nc()
    in_maps = _host_inputs(x, x_noise, z, W1, b1, W2, b2, b3, b_r)
    trace = bool(int(os.environ.get("KERNEL_TRACE", "0")))
    res = run_bass_kernel_spmd(nc, in_maps, core_ids=list(range(NCORES)),
                               trace=trace)
    LAST_RESULT = res
    results = res.results

    recover = np.concatenate([results[c]["out_r"] for c in range(NCORES)], axis=0)
    c2 = np.concatenate([results[c]["out_c2t"].T for c in range(NCORES)], axis=0)

    jacs = []
    for which in range(3):  # x, x_noise, z
        J = np.empty((B, D, D), dtype=np.float32)
        for c in range(NCORES):
            blk = np.asarray(results[c]["out_j"]).astype(np.float32)
            blk = blk.reshape(3, BS, D, D)[which]
            J[c * BS:(c + 1) * BS] = blk
        # reference's cat(dim=1)+reshape batch scrambling
        J = J.transpose(1, 0, 2).reshape(D, B * D).reshape(B, D, D)
        jacs.append(J)

    return (recover, c2, jacs[0], jacs[1], jacs[2])
